# revision 16
# baseline (speedup 1.0000x reference)
"""Trainium2 Bass kernel for a 2-layer dense-graph GAT (nn_GAT_79224966742097).

Reference (per batch sample, n=2048 nodes):
  layer0: x[2048,64] -> instance_norm over nodes -> 4-head GAT (f_out=64)
          -> gelu(concat heads + bias0) -> [2048, 256]
  layer1: instance_norm -> 1-head GAT (f_in=256, f_out=64) -> + bias1

Sharding: data-parallel over batch (bs=8) across the 8 NeuronCores; weights
replicated.  All tensors stay in SBUF; the n x n attention is never in HBM.

Attention layout: logits are computed TRANSPOSED, z^T[m, n] (m = softmax
reduction index on partitions) via z^T = a_sum-slice (stationary) @ hpT
(moving), where a_sum = a_src + a_dst is folded on host (the reference's
s+d terms share hp).  E = exp(leaky_relu(z^T)) and then
    U[n, o], denom[n] = sum_m E[m, n] * [hp | 1][m, o]
on the PE (K=128 accumulation), so the softmax denominator falls out of the
matmul's ones column -- no cross-partition reduction, no transpose of E.
Softmax max-subtraction is skipped: logits are bounded (|z| < 16) for
instance-normalized inputs, so exp stays far from fp32 limits.

Perf notes:
 * fp32r matmuls throughout: full PE rate at moving-dim 512 (the PSUM bank
   limit caps matmul free size at 512 fp32, so 16-bit moving buys nothing;
   and the DVE mis-converts PSUM->f16 on HW, so fp32 PSUM copies it is).
 * leaky-relu split between ACT (Prelu, same LUT set as Exp) and DVE
   (copy + in-place max(0.2*z, z)) to balance the elementwise engines;
   exp is ACT-only and dominates it.
 * softmax denominators: reciprocal_approx_fast (~18 bits, plenty for the
   2e-2 gate) instead of the slow DVE iterative divide, and U is copied
   PSUM->SBUF immediately so the PSUM accumulator frees for the next pass.
 * gelu + norm stats run per head-pair as soon as its two n-halves finish,
   overlapping the layer boundary with the attention stream.
 * `repeat` is a hardware For_i loop (not unrolled), so the NEFF stays
   small and timing runs amortize cleanly.
"""

import numpy as np

import concourse.bass as bass
import concourse.bacc as bacc
import concourse.mybir as mybir
import concourse.tile as tile
from contextlib import ExitStack
from concourse.masks import make_identity

F32 = mybir.dt.float32
F32R = mybir.dt.float32r
AX = mybir.AluOpType
ET = mybir.EngineType

N = 2048          # nodes
F0 = 64           # layer0 f_in
H0 = 4            # layer0 heads
FO = 64           # f_out (both layers)
NT = N // 128     # 16 node tiles
EPS = 1e-5
NEG_SLOPE = 0.2
N_CORES = 8

# lrelu engine split: m-unit t uses ACT-Prelu when (t % DEN) < NUM,
# else DVE copy + in-place max(0.2*z, z).
ACT_LRELU_NUM = 2
ACT_LRELU_DEN = 4
EB = 4            # m-units per exp batch


def _mmr(nc, out, lhsT, rhs, start=True, stop=True):
    nc.tensor.matmul(out, lhsT.bitcast(F32R), rhs.bitcast(F32R),
                     start=start, stop=stop)


def build_bass(sim_safe=False, act_lrelu=None, repeat=1, debug_taps=False):
    """Emit the full SPMD program for one core. Returns compiled nc."""
    nc = bacc.Bacc("TRN2", debug=False)

    x_d = nc.dram_tensor("x", [N, F0], F32, kind="ExternalInput")
    a0_d = nc.dram_tensor("a0", [128, 2, N], F32, kind="ExternalInput")
    w0_d = nc.dram_tensor("w0", [64, 2, 128], F32, kind="ExternalInput")
    b0_d = nc.dram_tensor("b0", [64, 1], F32, kind="ExternalInput")
    a1_d = nc.dram_tensor("a1", [128, N], F32, kind="ExternalInput")
    w1_d = nc.dram_tensor("w1", [64, 4, 128], F32, kind="ExternalInput")
    out_d = nc.dram_tensor("out", [N, FO], F32, kind="ExternalOutput")
    taps = None
    if debug_taps:
        taps = {k: nc.dram_tensor(k, s, F32, kind="ExternalOutput")
                for k, s in [("d_xT", [64, N]), ("d_hpT", [128, 2, N]),
                             ("d_hp1", [128, H0, NT, 65]),
                             ("d_x1g", [64, H0, N]),
                             ("d_x1T", [64, H0, N]), ("d_hpT1", [128, N]),
                             ("d_hp11", [128, NT, 65])]}

    gelu_func = (mybir.ActivationFunctionType.Identity if sim_safe
                 else mybir.ActivationFunctionType.Gelu)
    lrelu_func = mybir.ActivationFunctionType.Prelu
    if act_lrelu is None:
        act_lrelu = (ACT_LRELU_NUM, ACT_LRELU_DEN)

    with tile.TileContext(nc) as tc, ExitStack() as ctx:
        const = ctx.enter_context(tc.tile_pool(name="const", bufs=1))
        sb = ctx.enter_context(tc.tile_pool(name="sb", bufs=1))
        ps = ctx.enter_context(tc.tile_pool(name="ps", bufs=2, space="PSUM"))
        ep = ctx.enter_context(tc.tile_pool(name="ep", bufs=2))
        small = ctx.enter_context(tc.tile_pool(name="small", bufs=2))
        dram = ctx.enter_context(tc.tile_pool(name="dram", bufs=2, space="DRAM"))

        args = (nc, tc, const, sb, ps, ep, small, dram, sim_safe,
                act_lrelu, gelu_func, lrelu_func,
                x_d, a0_d, w0_d, b0_d, a1_d, w1_d, out_d, taps)
        if repeat == 1:
            body(*args)
        else:
            with tc.For_i(0, repeat, 1,
                          hint_engines=(ET.PE, ET.Activation, ET.DVE,
                                        ET.Pool, ET.SP)):
                body(*args)
    nc.compile()
    return nc


def body(nc, tc, const, sb, ps, ep, small, dram, sim_safe, act_lrelu,
         gelu_func, lrelu_func, x_d, a0_d, w0_d, b0_d, a1_d, w1_d, out_d,
         taps=None):
        ident = const.tile([128, 128], F32, name="ident", uniquify=True)
        make_identity(nc, ident)
        eps_sb = const.tile([128, 1], F32)
        nc.vector.memset(eps_sb, EPS)
        ones_sb = const.tile([128, 64], F32)
        nc.vector.memset(ones_sb, 1.0)

        b0_sb = const.tile([64, 1], F32)
        nc.sync.dma_start(out=b0_sb, in_=b0_d.ap())
        w0_sb = const.tile([64, 2, 128], F32)
        nc.sync.dma_start(out=w0_sb.bitcast(F32R), in_=w0_d.ap().bitcast(F32R))
        w1_sb = const.tile([64, 4, 128], F32)
        nc.sync.dma_start(out=w1_sb.bitcast(F32R), in_=w1_d.ap().bitcast(F32R))
        a0_sb = sb.tile([128, 2, N], F32)
        nc.sync.dma_start(out=a0_sb.bitcast(F32R), in_=a0_d.ap().bitcast(F32R))
        a1_sb = sb.tile([128, N], F32)
        nc.sync.dma_start(out=a1_sb.bitcast(F32R), in_=a1_d.ap().bitcast(F32R))

        # ---------- load x, transpose to [f, n], instance-norm ----------
        x_nt = ep.tile([128, NT, F0], F32, tag="e")
        nc.sync.dma_start(out=x_nt,
                          in_=x_d.ap().rearrange("(t p) f -> p t f", p=128))
        xT = sb.tile([64, N], F32)   # becomes x_normT in place
        xt_ps = ps.tile([64, N], F32, tag="z", bufs=1)
        for t in range(NT):
            nc.tensor.transpose(xt_ps[:, t * 128:(t + 1) * 128],
                                x_nt[:, t, :], ident)
        nc.vector.tensor_copy(out=xT.bitcast(F32R), in_=xt_ps)

        st0 = small.tile([64, 4, 6], F32, tag="st")
        for c in range(4):
            nc.vector.bn_stats(out=st0[:, c, :], in_=xT[:, c * 512:(c + 1) * 512])
        mv0 = small.tile([64, 2], F32, tag="mv")
        nc.vector.bn_aggr(out=mv0, in_=st0)
        sd0 = small.tile([64, 1], F32, tag="sd")
        nc.scalar.activation(out=sd0, in_=mv0[:, 1:2],
                             func=mybir.ActivationFunctionType.Sqrt,
                             bias=eps_sb[0:64, :])
        rs0 = small.tile([64, 1], F32, tag="rs")
        nc.vector.reciprocal(out=rs0, in_=sd0)
        nc.vector.tensor_scalar(out=xT.bitcast(F32R), in0=xT, scalar1=mv0[:, 0:1],
                                scalar2=rs0, op0=AX.subtract, op1=AX.mult)

        if taps:
            nc.sync.dma_start(out=taps["d_xT"].ap(), in_=xT)
        # ---------- layer0 h': hpT packed [128, j, n] + hp rows ----------
        # head h lives at partitions 64*(h%2)..+64, pair j = h//2
        hpT = sb.tile([128, 2, N], F32)
        for j in range(2):
            hp_ps = ps.tile([128, 2048], F32, tag="z", bufs=1, name="hp_ps")
            for c in range(4):
                _mmr(nc, hp_ps[:, c * 512:(c + 1) * 512],
                     w0_sb[:, j, :], xT[:, c * 512:(c + 1) * 512])
            nc.vector.tensor_copy(out=hpT[:, j, :].bitcast(F32R), in_=hp_ps)

        hp1 = sb.tile([128, H0, NT, 65], F32)
        nc.vector.tensor_copy(out=hp1[:, :, :, 64].bitcast(F32R),
                              in_=ones_sb.rearrange("p (a b) -> p a b", a=H0))
        w0cat = w0_sb.rearrange("p a b -> p (a b)")   # [64, 256] all heads
        for g in range(4):
            hpr_ps = ps.tile([128, 4, 256], F32, tag="z", bufs=1)
            for k in range(4):
                m = 4 * g + k
                _mmr(nc, hpr_ps[:, k, :],
                     xT[:, m * 128:(m + 1) * 128], w0cat)
            nc.vector.tensor_copy(
                out=hp1[:, :, 4 * g:4 * g + 4, 0:64].bitcast(F32R),
                in_=hpr_ps.rearrange("p m (h o) -> p h m o", h=H0))

        if taps:
            nc.sync.dma_start(out=taps["d_hpT"].ap(), in_=hpT)
            nc.sync.dma_start(out=taps["d_hp1"].ap(), in_=hp1)
        # ---------- attention block (shared by both layers) ----------
        lrelu_ctr = [0]

        def lrelu_on_act():
            t = lrelu_ctr[0]
            lrelu_ctr[0] += 1
            if sim_safe:
                return False
            return (t % act_lrelu[1]) < act_lrelu[0]

        def attention(z_mms, u_mms, n_m, finish):
            """Generic fused z -> lrelu -> exp -> U loop.  Each m-unit is a
            [128, 2048] z psum tile; lrelu lands in the fp32 et batch and
            exp runs in place, feeding the U matmuls."""
            zs = {}

            def emit_z(m):
                zt = ps.tile([128, 2048], F32, tag="z", bufs=1, name="zt")
                z_mms(m, zt)
                zs[m] = zt

            ets = {}

            def process(m):
                if m % EB == 0:
                    ets[m // EB] = ep.tile([128, EB * 2048], F32, tag="e",
                                           name="et")
                et = ets[m // EB]
                base = (m % EB) * 2048
                zt = zs.pop(m)
                dst = et[:, base:base + 2048]
                if lrelu_on_act():
                    nc.scalar.activation(out=dst.bitcast(F32R), in_=zt,
                                         func=lrelu_func, alpha=NEG_SLOPE)
                else:
                    nc.vector.tensor_copy(out=dst.bitcast(F32R), in_=zt)
                    nc.vector.scalar_tensor_tensor(
                        out=dst.bitcast(F32R), in0=dst, scalar=NEG_SLOPE,
                        in1=dst, op0=AX.mult, op1=AX.max)
                if m % EB == EB - 1:
                    nc.scalar.activation(out=et.bitcast(F32R), in_=et,
                                         func=mybir.ActivationFunctionType.Exp)
                    for mu in range(m - EB + 1, m + 1):
                        b2 = (mu % EB) * 2048
                        u_mms(mu, et[:, b2:b2 + 2048],
                              mu == 0, mu == n_m - 1)
                    ets.pop(m // EB)

            emit_z(0)
            for m in range(n_m):
                if m + 1 < n_m:
                    emit_z(m + 1)
                process(m)
            finish()

        # ---------- layer0 attention -> x1T [64, h, n] (pre-gelu) ----------
        x1T = sb.tile([64, H0, N], F32)

        st1 = small.tile([64, 4, 6], F32, tag="st")
        mv1 = small.tile([64, H0, 2], F32, tag="mv1")

        for j in range(2):
            for half in range(2):
                # per-head U accumulators [65, 1024] (2 PSUM banks each)
                u_ps = [ps.tile([65, 1024], F32, tag="u", bufs=2,
                                name=f"u{j}{half}{s}") for s in range(2)]

                def z_mms(m, zt, j=j, half=half):
                    for s in range(2):
                        for q in range(2):
                            n_off = half * 1024 + q * 512
                            _mmr(nc, zt[:, s * 1024 + q * 512:
                                        s * 1024 + (q + 1) * 512],
                                 a0_sb[64 * s:64 * s + 64, j,
                                       m * 128:(m + 1) * 128],
                                 hpT[64 * s:64 * s + 64, j,
                                     n_off:n_off + 512])

                def u_mms(m, et_sl, start, stop, j=j, u_ps=u_ps):
                    for s in range(2):
                        for q in range(2):
                            _mmr(nc, u_ps[s][:, q * 512:(q + 1) * 512],
                                 hp1[:, 2 * j + s, m, :],
                                 et_sl[:, s * 1024 + q * 512:
                                       s * 1024 + (q + 1) * 512],
                                 start=start, stop=stop)

                def finish(j=j, half=half, u_ps=u_ps):
                    # free the PSUM accumulators right away, then divide.
                    # The raw denominator row is DMA-broadcast to all 64
                    # partitions and reciprocated there: the custom DVE
                    # recip op only works partition-0-based on HW.
                    u_sb = ep.tile([65, 2, 1024], F32, tag="u_sb", bufs=1)
                    for s in range(2):
                        nc.vector.tensor_copy(out=u_sb[:, s, :], in_=u_ps[s])
                    r1_dram = dram.tile([1, 2048], F32, tag="r1d")
                    nc.sync.dma_start(
                        out=r1_dram,
                        in_=u_sb[64:65, :, :].rearrange("p a b -> p (a b)"))
                    rb_sb = ep.tile([64, 2, 1024], F32, tag="e")
                    bc_ap = bass.AP(tensor=r1_dram.tensor,
                                    offset=r1_dram.offset,
                                    ap=[[0, 64], [1, 2048]])
                    nc.sync.dma_start(
                        out=rb_sb.rearrange("p a b -> p (a b)"), in_=bc_ap)
                    nc.vector.reciprocal_approx_fast(
                        out=rb_sb.rearrange("p a b -> p (a b)"),
                        in_=rb_sb.rearrange("p a b -> p (a b)"))
                    dst = x1T[:, 2 * j:2 * j + 2,
                              half * 1024:(half + 1) * 1024]
                    nc.vector.tensor_mul(out=dst.bitcast(F32R),
                                         in0=u_sb[0:64, :, :], in1=rb_sb)

                attention(z_mms, u_mms, NT, finish)

            # gelu(x + bias0) for this head pair; overlap stats with the
            # next pair's attention stream.
            nc.scalar.activation(out=x1T[:, 2 * j:2 * j + 2, :].bitcast(F32R),
                                 in_=x1T[:, 2 * j:2 * j + 2, :],
                                 func=gelu_func, bias=b0_sb)
            for h in (2 * j, 2 * j + 1):
                for c in range(4):
                    nc.vector.bn_stats(out=st1[:, c, :],
                                       in_=x1T[:, h, c * 512:(c + 1) * 512])
                nc.vector.bn_aggr(out=mv1[:, h, :], in_=st1)

        if taps:
            nc.sync.dma_start(out=taps["d_x1g"].ap(), in_=x1T)
        # ---------- instance norm 1 (per channel = (h, o)) ----------
        sd1 = small.tile([64, H0], F32, tag="sd1")
        rs1 = small.tile([64, H0], F32, tag="rs1")
        nc.scalar.activation(out=sd1, in_=mv1[:, :, 1],
                             func=mybir.ActivationFunctionType.Sqrt,
                             bias=eps_sb[0:64, :])
        nc.vector.reciprocal(out=rs1, in_=sd1)
        for h in range(H0):
            nc.vector.tensor_scalar(out=x1T[:, h, :].bitcast(F32R),
                                    in0=x1T[:, h, :],
                                    scalar1=mv1[:, h, 0:1],
                                    scalar2=rs1[:, h:h + 1],
                                    op0=AX.subtract, op1=AX.mult)

        # ---------- layer1 h': hpT1 duplicated on both partition halves ----
        hpT1 = sb.tile([128, N], F32)
        hp_ps1 = ps.tile([128, 2048], F32, tag="z", bufs=1)
        for c in range(4):
            for kh in range(4):
                _mmr(nc, hp_ps1[:, c * 512:(c + 1) * 512],
                     w1_sb[:, kh, :], x1T[:, kh, c * 512:(c + 1) * 512],
                     start=(kh == 0), stop=(kh == 3))
        nc.vector.tensor_copy(out=hpT1.bitcast(F32R), in_=hp_ps1)

        hp11 = sb.tile([128, NT, 65], F32)
        nc.vector.tensor_copy(out=hp11[:, :, 64].bitcast(F32R),
                              in_=ones_sb[:, 0:NT])
        for g in range(2):
            hpr_ps1 = ps.tile([128, 8, FO], F32, tag="z", bufs=1)
            for k in range(8):
                m = 8 * g + k
                nc.tensor.transpose(hpr_ps1[:, k, :],
                                    hpT1[0:64, m * 128:(m + 1) * 128],
                                    ident[0:64, 0:64])
            nc.vector.tensor_copy(
                out=hp11[:, 8 * g:8 * g + 8, 0:64].bitcast(F32R),
                in_=hpr_ps1)

        if taps:
            nc.sync.dma_start(out=taps["d_x1T"].ap(), in_=x1T)
            nc.sync.dma_start(out=taps["d_hpT1"].ap(), in_=hpT1)
            nc.sync.dma_start(out=taps["d_hp11"].ap(), in_=hp11)
        # ---------- layer1 attention (m-tile pairs) -> out ----------
        out_sb = sb.tile([128, NT, FO], F32)

        for half in range(2):
            u1_ps = ps.tile([65, 1024], F32, tag="u", bufs=2)

            def z_mms1(m2, zt, half=half):
                for s in range(2):
                    m = 2 * m2 + s
                    for q in range(2):
                        n_off = half * 1024 + q * 512
                        _mmr(nc, zt[:, s * 1024 + q * 512:
                                    s * 1024 + (q + 1) * 512],
                             a1_sb[64 * s:64 * s + 64, m * 128:(m + 1) * 128],
                             hpT1[64 * s:64 * s + 64, n_off:n_off + 512])

            def u_mms1(m2, et_sl, start, stop, u1_ps=u1_ps):
                for s in range(2):
                    for q in range(2):
                        _mmr(nc, u1_ps[:, q * 512:(q + 1) * 512],
                             hp11[:, 2 * m2 + s, :],
                             et_sl[:, s * 1024 + q * 512:
                                   s * 1024 + (q + 1) * 512],
                             start=(start and s == 0), stop=(stop and s == 1))

            def finish1(half=half, u1_ps=u1_ps):
                u1 = ep.tile([65, 1024], F32, tag="u_sb", bufs=1)
                nc.vector.tensor_copy(out=u1, in_=u1_ps)
                for g in range(2):
                    tr_ps = ps.tile([128, 4, 65], F32, tag="u", bufs=2)
                    for k in range(4):
                        t = 4 * g + k
                        nc.tensor.transpose(tr_ps[:, k, :],
                                            u1[:, t * 128:(t + 1) * 128],
                                            ident[0:65, 0:65])
                    rr = small.tile([128, 4], F32, tag="rr")
                    nc.vector.reciprocal(out=rr, in_=tr_ps[:, :, 64])
                    for k in range(4):
                        t = half * 8 + 4 * g + k
                        nc.vector.tensor_scalar(
                            out=out_sb[:, t, :], in0=tr_ps[:, k, 0:64],
                            scalar1=rr[:, k:k + 1], scalar2=None,
                            op0=AX.mult)

            attention(z_mms1, u_mms1, NT // 2, finish1)

        nc.sync.dma_start(out=out_d.ap().rearrange("(t p) f -> p t f", p=128),
                          in_=out_sb)


def _prep_host(inputs):
    """Host-side packing of weights into device layouts (replicated)."""
    f32 = np.float32
    asum0 = (np.asarray(inputs['a_src0'], f32)
             + np.asarray(inputs['a_dst0'], f32))        # [4, 64, n]
    a0 = np.empty((128, 2, N), f32)
    for h in range(H0):
        a0[64 * (h % 2):64 * (h % 2) + 64, h // 2, :] = asum0[h]
    w0r = np.asarray(inputs['w0'], f32)                  # [4, 64, 64]
    w0 = np.empty((64, 2, 128), f32)
    for j in range(2):
        w0[:, j, 0:64] = w0r[2 * j]
        w0[:, j, 64:128] = w0r[2 * j + 1]
    b0 = np.ascontiguousarray(np.asarray(inputs['bias0'], f32).reshape(64, 1))
    asum1 = (np.asarray(inputs['a_src1'], f32)
             + np.asarray(inputs['a_dst1'], f32))[0]     # [64, n]
    a1 = np.concatenate([asum1, asum1], axis=0)          # [128, n] dup
    w1r = np.asarray(inputs['w1'], f32)[0].reshape(4, 64, FO)
    w1 = np.empty((64, 4, 128), f32)
    for kh in range(4):
        w1[:, kh, 0:64] = w1r[kh]
        w1[:, kh, 64:128] = w1r[kh]
    return {'a0': np.ascontiguousarray(a0),
            'w0': np.ascontiguousarray(w0), 'b0': b0,
            'a1': np.ascontiguousarray(a1),
            'w1': np.ascontiguousarray(w1)}


_NC_CACHE = {}


def _get_nc(sim_safe=False):
    if sim_safe not in _NC_CACHE:
        _NC_CACHE[sim_safe] = build_bass(sim_safe=sim_safe)
    return _NC_CACHE[sim_safe]


LAST_RESULTS = None  # BassKernelResults of the last kernel() call


def kernel(**inputs):
    from concourse.bass_utils import run_bass_kernel_spmd
    global LAST_RESULTS

    nc = _get_nc(sim_safe=False)
    w = _prep_host(inputs)
    x = np.asarray(inputs['x'], dtype=np.float32)
    in_maps = [{'x': np.ascontiguousarray(x[i]), **w} for i in range(N_CORES)]

    res = run_bass_kernel_spmd(nc, in_maps, core_ids=list(range(N_CORES)))
    LAST_RESULTS = res
    out = np.stack([res.results[i]['out'] for i in range(N_CORES)])
    out = out + np.asarray(inputs['bias1'], dtype=np.float32)[None, None, :]
    return out.astype(np.float32)


# revision 18
# speedup vs baseline: 4.8735x; 4.8735x over previous
"""Trainium2 Bass kernel for a 2-layer dense-graph GAT (nn_GAT_79224966742097).

Reference (per batch sample, n=2048 nodes):
  layer0: x[2048,64] -> instance_norm over nodes -> 4-head GAT (f_out=64)
          -> gelu(concat heads + bias0) -> [2048, 256]
  layer1: instance_norm -> 1-head GAT (f_in=256, f_out=64) -> + bias1

Sharding: data-parallel over batch (bs=8) across the 8 NeuronCores; weights
replicated.  All tensors stay in SBUF; the n x n attention is never in HBM.

Attention layout: logits are computed TRANSPOSED, z^T[m, n] (m = softmax
reduction index on partitions) via z^T = a_sum-slice (stationary) @ hpT
(moving), where a_sum = a_src + a_dst is folded on host (the reference's
s+d terms share hp).  E = exp(leaky_relu(z^T)) and then
    U[n, o], denom[n] = sum_m E[m, n] * [hp | 1][m, o]
on the PE (K=128 accumulation), so the softmax denominator falls out of the
matmul's ones column -- no cross-partition reduction, no transpose of E.
Softmax max-subtraction is skipped: logits are bounded (|z| < 16) for
instance-normalized inputs, so exp stays far from fp32 limits.

Perf notes:
 * fp32r matmuls throughout: full PE rate at moving-dim 512 (the PSUM bank
   limit caps matmul free size at 512 fp32, so 16-bit moving buys nothing;
   and the DVE mis-converts PSUM->f16 on HW, so fp32 PSUM copies it is).
 * leaky-relu split between ACT (Prelu, same LUT set as Exp) and DVE
   (copy + in-place max(0.2*z, z)) to balance the elementwise engines;
   exp is ACT-only and dominates it.
 * softmax denominators: reciprocal_approx_fast (~18 bits, plenty for the
   2e-2 gate) instead of the slow DVE iterative divide, and U is copied
   PSUM->SBUF immediately so the PSUM accumulator frees for the next pass.
 * gelu + norm stats run per head-pair as soon as its two n-halves finish,
   overlapping the layer boundary with the attention stream.
 * `repeat` is a hardware For_i loop (not unrolled), so the NEFF stays
   small and timing runs amortize cleanly.
"""

import numpy as np

import concourse.bass as bass
import concourse.bacc as bacc
import concourse.mybir as mybir
import concourse.tile as tile
from contextlib import ExitStack
from concourse.masks import make_identity

F32 = mybir.dt.float32
F32R = mybir.dt.float32r
AX = mybir.AluOpType
ET = mybir.EngineType

N = 2048          # nodes
F0 = 64           # layer0 f_in
H0 = 4            # layer0 heads
FO = 64           # f_out (both layers)
NT = N // 128     # 16 node tiles
EPS = 1e-5
NEG_SLOPE = 0.2
N_CORES = 8

# lrelu engine split: m-unit t uses ACT-Prelu when (t % DEN) < NUM,
# else DVE copy + in-place max(0.2*z, z).
ACT_LRELU_NUM = 2
ACT_LRELU_DEN = 4
EB = 4            # m-units per exp batch


def _mmr(nc, out, lhsT, rhs, start=True, stop=True):
    nc.tensor.matmul(out, lhsT.bitcast(F32R), rhs.bitcast(F32R),
                     start=start, stop=stop)


def build_bass(sim_safe=False, act_lrelu=None, repeat=1, debug_taps=False):
    """Emit the full SPMD program for one core. Returns compiled nc."""
    nc = bacc.Bacc("TRN2", debug=False)

    x_d = nc.dram_tensor("x", [N, F0], F32, kind="ExternalInput")
    a0_d = nc.dram_tensor("a0", [128, 2, N], F32, kind="ExternalInput")
    w0_d = nc.dram_tensor("w0", [64, 2, 128], F32, kind="ExternalInput")
    b0_d = nc.dram_tensor("b0", [64, 1], F32, kind="ExternalInput")
    a1_d = nc.dram_tensor("a1", [128, N], F32, kind="ExternalInput")
    w1_d = nc.dram_tensor("w1", [64, 4, 128], F32, kind="ExternalInput")
    out_d = nc.dram_tensor("out", [N, FO], F32, kind="ExternalOutput")
    taps = None
    if debug_taps:
        taps = {k: nc.dram_tensor(k, s, F32, kind="ExternalOutput")
                for k, s in [("d_xT", [64, N]), ("d_hpT", [128, 2, N]),
                             ("d_hp1", [128, H0, NT, 65]),
                             ("d_x1g", [64, H0, N]),
                             ("d_x1T", [64, H0, N]), ("d_hpT1", [128, N]),
                             ("d_hp11", [128, NT, 65])]}

    gelu_func = (mybir.ActivationFunctionType.Identity if sim_safe
                 else mybir.ActivationFunctionType.Gelu)
    lrelu_func = mybir.ActivationFunctionType.Prelu
    if act_lrelu is None:
        act_lrelu = (ACT_LRELU_NUM, ACT_LRELU_DEN)

    with tile.TileContext(nc) as tc, ExitStack() as ctx:
        const = ctx.enter_context(tc.tile_pool(name="const", bufs=1))
        sb = ctx.enter_context(tc.tile_pool(name="sb", bufs=1))
        ps = ctx.enter_context(tc.tile_pool(name="ps", bufs=2, space="PSUM"))
        ep = ctx.enter_context(tc.tile_pool(name="ep", bufs=2))
        small = ctx.enter_context(tc.tile_pool(name="small", bufs=2))
        dram = ctx.enter_context(tc.tile_pool(name="dram", bufs=2, space="DRAM"))

        args = (nc, tc, const, sb, ps, ep, small, dram, sim_safe,
                act_lrelu, gelu_func, lrelu_func,
                x_d, a0_d, w0_d, b0_d, a1_d, w1_d, out_d, taps)
        if repeat == 1:
            body(*args)
        else:
            with tc.For_i(0, repeat, 1,
                          hint_engines=(ET.PE, ET.Activation, ET.DVE,
                                        ET.Pool, ET.SP)):
                body(*args)
    nc.compile()
    return nc


def body(nc, tc, const, sb, ps, ep, small, dram, sim_safe, act_lrelu,
         gelu_func, lrelu_func, x_d, a0_d, w0_d, b0_d, a1_d, w1_d, out_d,
         taps=None):
        ident = const.tile([128, 128], F32, name="ident", uniquify=True)
        make_identity(nc, ident)
        eps_sb = const.tile([128, 1], F32)
        nc.vector.memset(eps_sb, EPS)
        ones_sb = const.tile([128, 64], F32)
        nc.vector.memset(ones_sb, 1.0)

        b0_sb = const.tile([64, 1], F32)
        nc.sync.dma_start(out=b0_sb, in_=b0_d.ap())
        w0_sb = const.tile([64, 2, 128], F32)
        nc.sync.dma_start(out=w0_sb.bitcast(F32R), in_=w0_d.ap().bitcast(F32R))
        w1_sb = const.tile([64, 4, 128], F32)
        nc.sync.dma_start(out=w1_sb.bitcast(F32R), in_=w1_d.ap().bitcast(F32R))
        a0_sb = sb.tile([128, 2, N], F32)
        nc.sync.dma_start(out=a0_sb.bitcast(F32R), in_=a0_d.ap().bitcast(F32R))
        a1_sb = sb.tile([128, N], F32)
        nc.sync.dma_start(out=a1_sb.bitcast(F32R), in_=a1_d.ap().bitcast(F32R))

        # ---------- load x, transpose to [f, n], instance-norm ----------
        x_nt = ep.tile([128, NT, F0], F32, tag="xn", bufs=1)
        nc.sync.dma_start(out=x_nt,
                          in_=x_d.ap().rearrange("(t p) f -> p t f", p=128))
        xT = sb.tile([64, N], F32)   # becomes x_normT in place
        xt_ps = ps.tile([64, N], F32, tag="z", bufs=1)
        for t in range(NT):
            nc.tensor.transpose(xt_ps[:, t * 128:(t + 1) * 128],
                                x_nt[:, t, :], ident)
        nc.vector.tensor_copy(out=xT.bitcast(F32R), in_=xt_ps)

        st0 = small.tile([64, 4, 6], F32, tag="st")
        for c in range(4):
            nc.vector.bn_stats(out=st0[:, c, :], in_=xT[:, c * 512:(c + 1) * 512])
        mv0 = small.tile([64, 2], F32, tag="mv")
        nc.vector.bn_aggr(out=mv0, in_=st0)
        sd0 = small.tile([64, 1], F32, tag="sd")
        nc.scalar.activation(out=sd0, in_=mv0[:, 1:2],
                             func=mybir.ActivationFunctionType.Sqrt,
                             bias=eps_sb[0:64, :])
        rs0 = small.tile([64, 1], F32, tag="rs")
        nc.vector.reciprocal(out=rs0, in_=sd0)
        nc.vector.tensor_scalar(out=xT.bitcast(F32R), in0=xT, scalar1=mv0[:, 0:1],
                                scalar2=rs0, op0=AX.subtract, op1=AX.mult)

        if taps:
            nc.sync.dma_start(out=taps["d_xT"].ap(), in_=xT)
        # ---------- layer0 h': hpT packed [128, j, n] + hp rows ----------
        # head h lives at partitions 64*(h%2)..+64, pair j = h//2
        hpT = sb.tile([128, 2, N], F32)
        for j in range(2):
            hp_ps = ps.tile([128, 2048], F32, tag="z", bufs=1, name="hp_ps")
            for c in range(4):
                _mmr(nc, hp_ps[:, c * 512:(c + 1) * 512],
                     w0_sb[:, j, :], xT[:, c * 512:(c + 1) * 512])
            nc.vector.tensor_copy(out=hpT[:, j, :].bitcast(F32R), in_=hp_ps)

        hp1 = sb.tile([128, H0, NT, 65], F32)
        nc.vector.tensor_copy(out=hp1[:, :, :, 64].bitcast(F32R),
                              in_=ones_sb.rearrange("p (a b) -> p a b", a=H0))
        w0cat = w0_sb.rearrange("p a b -> p (a b)")   # [64, 256] all heads
        for g in range(4):
            hpr_ps = ps.tile([128, 4, 256], F32, tag="z", bufs=1)
            for k in range(4):
                m = 4 * g + k
                _mmr(nc, hpr_ps[:, k, :],
                     xT[:, m * 128:(m + 1) * 128], w0cat)
            nc.vector.tensor_copy(
                out=hp1[:, :, 4 * g:4 * g + 4, 0:64].bitcast(F32R),
                in_=hpr_ps.rearrange("p m (h o) -> p h m o", h=H0))

        if taps:
            nc.sync.dma_start(out=taps["d_hpT"].ap(), in_=hpT)
            nc.sync.dma_start(out=taps["d_hp1"].ap(), in_=hp1)
        # ---------- attention block (shared by both layers) ----------
        lrelu_ctr = [0]

        def lrelu_on_act():
            t = lrelu_ctr[0]
            lrelu_ctr[0] += 1
            if sim_safe:
                return False
            return (t % act_lrelu[1]) < act_lrelu[0]

        def attention(z_mms, u_mms, n_m, finish):
            """Generic fused z -> lrelu -> exp -> U loop.  Each m-unit is a
            [128, 2048] z psum tile; lrelu lands in the fp32 et batch and
            exp runs in place, feeding the U matmuls."""
            zs = {}

            def emit_z(m):
                zt = ps.tile([128, 2048], F32, tag="z", bufs=1, name="zt")
                z_mms(m, zt)
                zs[m] = zt

            ets = {}

            def process(m):
                if m % EB == 0:
                    ets[m // EB] = ep.tile([128, EB * 2048], F32, tag="e",
                                           name="et")
                et = ets[m // EB]
                base = (m % EB) * 2048
                zt = zs.pop(m)
                dst = et[:, base:base + 2048]
                if lrelu_on_act():
                    nc.scalar.activation(out=dst.bitcast(F32R), in_=zt,
                                         func=lrelu_func, alpha=NEG_SLOPE)
                else:
                    nc.vector.tensor_copy(out=dst.bitcast(F32R), in_=zt)
                    nc.vector.scalar_tensor_tensor(
                        out=dst.bitcast(F32R), in0=dst, scalar=NEG_SLOPE,
                        in1=dst, op0=AX.mult, op1=AX.max)
                if m % EB == EB - 1:
                    nc.scalar.activation(out=et.bitcast(F32R), in_=et,
                                         func=mybir.ActivationFunctionType.Exp)
                    for mu in range(m - EB + 1, m + 1):
                        b2 = (mu % EB) * 2048
                        u_mms(mu, et[:, b2:b2 + 2048],
                              mu == 0, mu == n_m - 1)
                    ets.pop(m // EB)

            emit_z(0)
            for m in range(n_m):
                if m + 1 < n_m:
                    emit_z(m + 1)
                process(m)
            finish()

        # ---------- layer0 attention -> x1T [64, h, n] (pre-gelu) ----------
        x1T = sb.tile([64, H0, N], F32)

        st1 = small.tile([64, 4, 6], F32, tag="st")
        mv1 = small.tile([64, H0, 2], F32, tag="mv1")

        for j in range(2):
            for half in range(2):
                # per-head U accumulators [65, 1024] (2 PSUM banks each)
                u_ps = [ps.tile([65, 1024], F32, tag="u", bufs=2,
                                name=f"u{j}{half}{s}") for s in range(2)]

                def z_mms(m, zt, j=j, half=half):
                    for s in range(2):
                        for q in range(2):
                            n_off = half * 1024 + q * 512
                            _mmr(nc, zt[:, s * 1024 + q * 512:
                                        s * 1024 + (q + 1) * 512],
                                 a0_sb[64 * s:64 * s + 64, j,
                                       m * 128:(m + 1) * 128],
                                 hpT[64 * s:64 * s + 64, j,
                                     n_off:n_off + 512])

                def u_mms(m, et_sl, start, stop, j=j, u_ps=u_ps):
                    for s in range(2):
                        for q in range(2):
                            _mmr(nc, u_ps[s][:, q * 512:(q + 1) * 512],
                                 hp1[:, 2 * j + s, m, :],
                                 et_sl[:, s * 1024 + q * 512:
                                       s * 1024 + (q + 1) * 512],
                                 start=start, stop=stop)

                def finish(j=j, half=half, u_ps=u_ps):
                    # denominator row goes PSUM -> DRAM -> 64-partition
                    # broadcast; reciprocal runs on the broadcast (the
                    # custom DVE recip only works partition-0-based on HW).
                    # U itself is copied out right away to free the PSUM
                    # accumulators for the next pass.
                    u_sb = ep.tile([65, 2, 1024], F32, tag="u_sb", bufs=1)
                    for s in range(2):
                        nc.vector.tensor_copy(out=u_sb[:, s, :], in_=u_ps[s])
                    r1_dram = dram.tile([1, 2, 1024], F32, tag="r1d")
                    nc.sync.dma_start(
                        out=r1_dram,
                        in_=u_sb[64:65, :, :])
                    rb_sb = ep.tile([64, 2, 1024], F32, tag="rb", bufs=1)
                    bc_ap = bass.AP(tensor=r1_dram.tensor,
                                    offset=r1_dram.offset,
                                    ap=[[0, 64], [1, 2048]])
                    nc.sync.dma_start(
                        out=rb_sb.rearrange("p a b -> p (a b)"), in_=bc_ap)
                    nc.vector.reciprocal_approx_fast(
                        out=rb_sb.rearrange("p a b -> p (a b)"),
                        in_=rb_sb.rearrange("p a b -> p (a b)"))
                    dst = x1T[:, 2 * j:2 * j + 2,
                              half * 1024:(half + 1) * 1024]
                    nc.vector.tensor_mul(out=dst.bitcast(F32R),
                                         in0=u_sb[0:64, :, :], in1=rb_sb)

                attention(z_mms, u_mms, NT, finish)

            # gelu(x + bias0) for this head pair; overlap stats with the
            # next pair's attention stream.
            nc.scalar.activation(out=x1T[:, 2 * j:2 * j + 2, :].bitcast(F32R),
                                 in_=x1T[:, 2 * j:2 * j + 2, :],
                                 func=gelu_func, bias=b0_sb)
            for h in (2 * j, 2 * j + 1):
                for c in range(4):
                    nc.vector.bn_stats(out=st1[:, c, :],
                                       in_=x1T[:, h, c * 512:(c + 1) * 512])
                nc.vector.bn_aggr(out=mv1[:, h, :], in_=st1)

        if taps:
            nc.sync.dma_start(out=taps["d_x1g"].ap(), in_=x1T)
        # ---------- instance norm 1 (per channel = (h, o)) ----------
        sd1 = small.tile([64, H0], F32, tag="sd1")
        rs1 = small.tile([64, H0], F32, tag="rs1")
        nc.scalar.activation(out=sd1, in_=mv1[:, :, 1],
                             func=mybir.ActivationFunctionType.Sqrt,
                             bias=eps_sb[0:64, :])
        nc.vector.reciprocal(out=rs1, in_=sd1)
        for h in range(H0):
            nc.vector.tensor_scalar(out=x1T[:, h, :].bitcast(F32R),
                                    in0=x1T[:, h, :],
                                    scalar1=mv1[:, h, 0:1],
                                    scalar2=rs1[:, h:h + 1],
                                    op0=AX.subtract, op1=AX.mult)

        # ---------- layer1 h': hpT1 duplicated on both partition halves ----
        hpT1 = sb.tile([128, N], F32)
        hp_ps1 = ps.tile([128, 2048], F32, tag="z", bufs=1)
        for kh in range(4):
            for c in range(4):
                _mmr(nc, hp_ps1[:, c * 512:(c + 1) * 512],
                     w1_sb[:, kh, :], x1T[:, kh, c * 512:(c + 1) * 512],
                     start=(kh == 0), stop=(kh == 3))
        nc.vector.tensor_copy(out=hpT1.bitcast(F32R), in_=hp_ps1)

        hp11 = sb.tile([128, NT, 65], F32)
        nc.vector.tensor_copy(out=hp11[:, :, 64].bitcast(F32R),
                              in_=ones_sb[:, 0:NT])
        for g in range(2):
            hpr_ps1 = ps.tile([128, 8, FO], F32, tag="z", bufs=1)
            for k in range(8):
                m = 8 * g + k
                nc.tensor.transpose(hpr_ps1[:, k, :],
                                    hpT1[0:64, m * 128:(m + 1) * 128],
                                    ident[0:64, 0:64])
            nc.vector.tensor_copy(
                out=hp11[:, 8 * g:8 * g + 8, 0:64].bitcast(F32R),
                in_=hpr_ps1)

        if taps:
            nc.sync.dma_start(out=taps["d_x1T"].ap(), in_=x1T)
            nc.sync.dma_start(out=taps["d_hpT1"].ap(), in_=hpT1)
            nc.sync.dma_start(out=taps["d_hp11"].ap(), in_=hp11)
        # ---------- layer1 attention (m-tile pairs) -> out ----------
        out_sb = sb.tile([128, NT, FO], F32)

        for half in range(2):
            u1_ps = ps.tile([65, 1024], F32, tag="u", bufs=2)

            def z_mms1(m2, zt, half=half):
                for s in range(2):
                    m = 2 * m2 + s
                    for q in range(2):
                        n_off = half * 1024 + q * 512
                        _mmr(nc, zt[:, s * 1024 + q * 512:
                                    s * 1024 + (q + 1) * 512],
                             a1_sb[64 * s:64 * s + 64, m * 128:(m + 1) * 128],
                             hpT1[64 * s:64 * s + 64, n_off:n_off + 512])

            def u_mms1(m2, et_sl, start, stop, u1_ps=u1_ps):
                for s in range(2):
                    for q in range(2):
                        _mmr(nc, u1_ps[:, q * 512:(q + 1) * 512],
                             hp11[:, 2 * m2 + s, :],
                             et_sl[:, s * 1024 + q * 512:
                                   s * 1024 + (q + 1) * 512],
                             start=(start and s == 0), stop=(stop and s == 1))

            def finish1(half=half, u1_ps=u1_ps):
                u1 = ep.tile([65, 1024], F32, tag="u_sb", bufs=1)
                nc.vector.tensor_copy(out=u1, in_=u1_ps)
                for g in range(2):
                    tr_ps = ps.tile([128, 4, 65], F32, tag="u", bufs=2)
                    for k in range(4):
                        t = 4 * g + k
                        nc.tensor.transpose(tr_ps[:, k, :],
                                            u1[:, t * 128:(t + 1) * 128],
                                            ident[0:65, 0:65])
                    rr = small.tile([128, 4], F32, tag="rr")
                    nc.vector.reciprocal(out=rr, in_=tr_ps[:, :, 64])
                    for k in range(4):
                        t = half * 8 + 4 * g + k
                        nc.vector.tensor_scalar(
                            out=out_sb[:, t, :], in0=tr_ps[:, k, 0:64],
                            scalar1=rr[:, k:k + 1], scalar2=None,
                            op0=AX.mult)

            attention(z_mms1, u_mms1, NT // 2, finish1)

        nc.sync.dma_start(out=out_d.ap().rearrange("(t p) f -> p t f", p=128),
                          in_=out_sb)


def _prep_host(inputs):
    """Host-side packing of weights into device layouts (replicated)."""
    f32 = np.float32
    asum0 = (np.asarray(inputs['a_src0'], f32)
             + np.asarray(inputs['a_dst0'], f32))        # [4, 64, n]
    a0 = np.empty((128, 2, N), f32)
    for h in range(H0):
        a0[64 * (h % 2):64 * (h % 2) + 64, h // 2, :] = asum0[h]
    w0r = np.asarray(inputs['w0'], f32)                  # [4, 64, 64]
    w0 = np.empty((64, 2, 128), f32)
    for j in range(2):
        w0[:, j, 0:64] = w0r[2 * j]
        w0[:, j, 64:128] = w0r[2 * j + 1]
    b0 = np.ascontiguousarray(np.asarray(inputs['bias0'], f32).reshape(64, 1))
    asum1 = (np.asarray(inputs['a_src1'], f32)
             + np.asarray(inputs['a_dst1'], f32))[0]     # [64, n]
    a1 = np.concatenate([asum1, asum1], axis=0)          # [128, n] dup
    w1r = np.asarray(inputs['w1'], f32)[0].reshape(4, 64, FO)
    w1 = np.empty((64, 4, 128), f32)
    for kh in range(4):
        w1[:, kh, 0:64] = w1r[kh]
        w1[:, kh, 64:128] = w1r[kh]
    return {'a0': np.ascontiguousarray(a0),
            'w0': np.ascontiguousarray(w0), 'b0': b0,
            'a1': np.ascontiguousarray(a1),
            'w1': np.ascontiguousarray(w1)}


_NC_CACHE = {}


def _get_nc(sim_safe=False):
    if sim_safe not in _NC_CACHE:
        _NC_CACHE[sim_safe] = build_bass(sim_safe=sim_safe)
    return _NC_CACHE[sim_safe]


LAST_RESULTS = None  # BassKernelResults of the last kernel() call


def kernel(**inputs):
    from concourse.bass_utils import run_bass_kernel_spmd
    global LAST_RESULTS

    nc = _get_nc(sim_safe=False)
    w = _prep_host(inputs)
    x = np.asarray(inputs['x'], dtype=np.float32)
    in_maps = [{'x': np.ascontiguousarray(x[i]), **w} for i in range(N_CORES)]

    res = run_bass_kernel_spmd(nc, in_maps, core_ids=list(range(N_CORES)))
    LAST_RESULTS = res
    out = np.stack([res.results[i]['out'] for i in range(N_CORES)])
    out = out + np.asarray(inputs['bias1'], dtype=np.float32)[None, None, :]
    return out.astype(np.float32)


# revision 19
# speedup vs baseline: 6.7162x; 1.3781x over previous
"""Trainium2 Bass kernel for a 2-layer dense-graph GAT (nn_GAT_79224966742097).

Reference (per batch sample, n=2048 nodes):
  layer0: x[2048,64] -> instance_norm over nodes -> 4-head GAT (f_out=64)
          -> gelu(concat heads + bias0) -> [2048, 256]
  layer1: instance_norm -> 1-head GAT (f_in=256, f_out=64) -> + bias1

Sharding: data-parallel over batch (bs=8) across the 8 NeuronCores; weights
replicated.  All tensors stay in SBUF; the n x n attention is never in HBM.

Attention layout: logits are computed TRANSPOSED, z^T[m, n] (m = softmax
reduction index on partitions) via z^T = a_sum-slice (stationary) @ hpT
(moving), where a_sum = a_src + a_dst is folded on host (the reference's
s+d terms share hp).  E = exp(leaky_relu(z^T)) and then
    U[n, o], denom[n] = sum_m E[m, n] * [hp | 1][m, o]
on the PE (K=128 accumulation), so the softmax denominator falls out of the
matmul's ones column -- no cross-partition reduction, no transpose of E.
Softmax max-subtraction is skipped: logits are bounded (|z| < 16) for
instance-normalized inputs, so exp stays far from fp32 limits.

Perf notes:
 * fp32r matmuls throughout: full PE rate at moving-dim 512 (the PSUM bank
   limit caps matmul free size at 512 fp32, so 16-bit moving buys nothing;
   and the DVE mis-converts PSUM->f16 on HW, so fp32 PSUM copies it is).
 * leaky-relu split between ACT (Prelu, same LUT set as Exp) and DVE
   (copy + in-place max(0.2*z, z)) to balance the elementwise engines;
   exp is ACT-only and dominates it.
 * softmax denominators: reciprocal_approx_fast (~18 bits, plenty for the
   2e-2 gate) instead of the slow DVE iterative divide, and U is copied
   PSUM->SBUF immediately so the PSUM accumulator frees for the next pass.
 * gelu + norm stats run per head-pair as soon as its two n-halves finish,
   overlapping the layer boundary with the attention stream.
 * `repeat` is a hardware For_i loop (not unrolled), so the NEFF stays
   small and timing runs amortize cleanly.
"""

import numpy as np

import concourse.bass as bass
import concourse.bacc as bacc
import concourse.mybir as mybir
import concourse.tile as tile
from contextlib import ExitStack
from concourse.masks import make_identity

F32 = mybir.dt.float32
F32R = mybir.dt.float32r
AX = mybir.AluOpType
ET = mybir.EngineType

N = 2048          # nodes
F0 = 64           # layer0 f_in
H0 = 4            # layer0 heads
FO = 64           # f_out (both layers)
NT = N // 128     # 16 node tiles
EPS = 1e-5
NEG_SLOPE = 0.2
N_CORES = 8

# lrelu engine split: m-unit t uses ACT-Prelu when (t % DEN) < NUM,
# else DVE copy + in-place max(0.2*z, z).
ACT_LRELU_NUM = 2
ACT_LRELU_DEN = 4
EB = 4            # m-units per exp batch


def _mmr(nc, out, lhsT, rhs, start=True, stop=True):
    nc.tensor.matmul(out, lhsT.bitcast(F32R), rhs.bitcast(F32R),
                     start=start, stop=stop)


def build_bass(sim_safe=False, act_lrelu=None, repeat=1, debug_taps=False):
    """Emit the full SPMD program for one core. Returns compiled nc."""
    nc = bacc.Bacc("TRN2", debug=False)

    x_d = nc.dram_tensor("x", [N, F0], F32, kind="ExternalInput")
    a0_d = nc.dram_tensor("a0", [128, 2, N], F32, kind="ExternalInput")
    w0_d = nc.dram_tensor("w0", [64, 2, 128], F32, kind="ExternalInput")
    b0_d = nc.dram_tensor("b0", [64, 1], F32, kind="ExternalInput")
    a1_d = nc.dram_tensor("a1", [128, N], F32, kind="ExternalInput")
    w1_d = nc.dram_tensor("w1", [64, 4, 128], F32, kind="ExternalInput")
    out_d = nc.dram_tensor("out", [N, FO], F32, kind="ExternalOutput")
    taps = None
    if debug_taps:
        taps = {k: nc.dram_tensor(k, s, F32, kind="ExternalOutput")
                for k, s in [("d_xT", [64, N]), ("d_hpT", [128, 2, N]),
                             ("d_hp1", [128, H0, NT, 65]),
                             ("d_x1g", [64, H0, N]),
                             ("d_x1T", [64, H0, N]), ("d_hpT1", [128, N]),
                             ("d_hp11", [128, NT, 65])]}

    gelu_func = (mybir.ActivationFunctionType.Identity if sim_safe
                 else mybir.ActivationFunctionType.Gelu)
    lrelu_func = mybir.ActivationFunctionType.Prelu
    if act_lrelu is None:
        act_lrelu = (ACT_LRELU_NUM, ACT_LRELU_DEN)

    with tile.TileContext(nc) as tc, ExitStack() as ctx:
        const = ctx.enter_context(tc.tile_pool(name="const", bufs=1))
        sb = ctx.enter_context(tc.tile_pool(name="sb", bufs=1))
        ps = ctx.enter_context(tc.tile_pool(name="ps", bufs=2, space="PSUM"))
        ep = ctx.enter_context(tc.tile_pool(name="ep", bufs=2))
        small = ctx.enter_context(tc.tile_pool(name="small", bufs=2))
        dram = ctx.enter_context(tc.tile_pool(name="dram", bufs=2, space="DRAM"))

        args = (nc, tc, const, sb, ps, ep, small, dram, sim_safe,
                act_lrelu, gelu_func, lrelu_func,
                x_d, a0_d, w0_d, b0_d, a1_d, w1_d, out_d, taps)
        if repeat == 1:
            body(*args)
        else:
            with tc.For_i(0, repeat, 1, staggered_reset=True,
                          hint_engines=(ET.PE, ET.Activation, ET.DVE,
                                        ET.Pool, ET.SP)):
                body(*args)
    nc.compile()
    return nc


def body(nc, tc, const, sb, ps, ep, small, dram, sim_safe, act_lrelu,
         gelu_func, lrelu_func, x_d, a0_d, w0_d, b0_d, a1_d, w1_d, out_d,
         taps=None):
        ident = const.tile([128, 128], F32, name="ident", uniquify=True)
        make_identity(nc, ident)
        eps_sb = const.tile([128, 1], F32)
        nc.vector.memset(eps_sb, EPS)
        ones_sb = const.tile([128, 64], F32)
        nc.vector.memset(ones_sb, 1.0)

        b0_sb = const.tile([64, 1], F32)
        nc.sync.dma_start(out=b0_sb, in_=b0_d.ap())
        w0_sb = const.tile([64, 2, 128], F32)
        nc.sync.dma_start(out=w0_sb.bitcast(F32R), in_=w0_d.ap().bitcast(F32R))
        w1_sb = const.tile([64, 4, 128], F32)
        nc.sync.dma_start(out=w1_sb.bitcast(F32R), in_=w1_d.ap().bitcast(F32R))
        a0_sb = sb.tile([128, 2, N], F32)
        nc.sync.dma_start(out=a0_sb.bitcast(F32R), in_=a0_d.ap().bitcast(F32R))
        a1_sb = sb.tile([128, N], F32)
        nc.sync.dma_start(out=a1_sb.bitcast(F32R), in_=a1_d.ap().bitcast(F32R))

        # ---------- load x, transpose to [f, n], instance-norm ----------
        x_nt = ep.tile([128, NT, F0], F32, tag="xn", bufs=1)
        nc.sync.dma_start(out=x_nt,
                          in_=x_d.ap().rearrange("(t p) f -> p t f", p=128))
        xT = sb.tile([64, N], F32)   # becomes x_normT in place
        xt_ps = ps.tile([64, N], F32, tag="z", bufs=1)
        for t in range(NT):
            nc.tensor.transpose(xt_ps[:, t * 128:(t + 1) * 128],
                                x_nt[:, t, :], ident)
        nc.vector.tensor_copy(out=xT.bitcast(F32R), in_=xt_ps)

        st0 = small.tile([64, 4, 6], F32, tag="st")
        for c in range(4):
            nc.vector.bn_stats(out=st0[:, c, :], in_=xT[:, c * 512:(c + 1) * 512])
        mv0 = small.tile([64, 2], F32, tag="mv")
        nc.vector.bn_aggr(out=mv0, in_=st0)
        sd0 = small.tile([64, 1], F32, tag="sd")
        nc.scalar.activation(out=sd0, in_=mv0[:, 1:2],
                             func=mybir.ActivationFunctionType.Sqrt,
                             bias=eps_sb[0:64, :])
        rs0 = small.tile([64, 1], F32, tag="rs")
        nc.vector.reciprocal(out=rs0, in_=sd0)
        nc.vector.tensor_scalar(out=xT.bitcast(F32R), in0=xT, scalar1=mv0[:, 0:1],
                                scalar2=rs0, op0=AX.subtract, op1=AX.mult)

        if taps:
            nc.sync.dma_start(out=taps["d_xT"].ap(), in_=xT)
        # ---------- layer0 h': hpT packed [128, j, n] + hp rows ----------
        # head h lives at partitions 64*(h%2)..+64, pair j = h//2
        hpT = sb.tile([128, 2, N], F32)
        for j in range(2):
            hp_ps = ps.tile([128, 2048], F32, tag="z", bufs=1, name="hp_ps")
            for c in range(4):
                _mmr(nc, hp_ps[:, c * 512:(c + 1) * 512],
                     w0_sb[:, j, :], xT[:, c * 512:(c + 1) * 512])
            nc.vector.tensor_copy(out=hpT[:, j, :].bitcast(F32R), in_=hp_ps)

        hp1 = sb.tile([128, H0, NT, 65], F32)
        nc.vector.tensor_copy(out=hp1[:, :, :, 64].bitcast(F32R),
                              in_=ones_sb.rearrange("p (a b) -> p a b", a=H0))
        w0cat = w0_sb.rearrange("p a b -> p (a b)")   # [64, 256] all heads
        for g in range(4):
            hpr_ps = ps.tile([128, 4, 256], F32, tag="z", bufs=1)
            for k in range(4):
                m = 4 * g + k
                _mmr(nc, hpr_ps[:, k, :],
                     xT[:, m * 128:(m + 1) * 128], w0cat)
            nc.vector.tensor_copy(
                out=hp1[:, :, 4 * g:4 * g + 4, 0:64].bitcast(F32R),
                in_=hpr_ps.rearrange("p m (h o) -> p h m o", h=H0))

        if taps:
            nc.sync.dma_start(out=taps["d_hpT"].ap(), in_=hpT)
            nc.sync.dma_start(out=taps["d_hp1"].ap(), in_=hp1)
        # ---------- attention block (shared by both layers) ----------
        lrelu_ctr = [0]

        def lrelu_on_act():
            t = lrelu_ctr[0]
            lrelu_ctr[0] += 1
            if sim_safe:
                return False
            # ACT takes the LAST units of each exp batch so exp(k) never
            # waits on the DVE lrelu tail (DVE runs ahead on batch k+1).
            return (t % act_lrelu[1]) >= act_lrelu[1] - act_lrelu[0]

        def attention(z_mms, u_mms, n_m, finish):
            """Generic fused z -> lrelu -> exp -> U loop.  Each m-unit is a
            [128, 2048] z psum tile; lrelu lands in the fp32 et batch and
            exp runs in place, feeding the U matmuls."""
            zs = {}

            def emit_z(m):
                zt = ps.tile([128, 2048], F32, tag="z", bufs=1, name="zt")
                z_mms(m, zt)
                zs[m] = zt

            ets = {}

            def process(m):
                if m % EB == 0:
                    ets[m // EB] = ep.tile([128, EB * 2048], F32, tag="e",
                                           name="et")
                et = ets[m // EB]
                base = (m % EB) * 2048
                zt = zs.pop(m)
                dst = et[:, base:base + 2048]
                if lrelu_on_act():
                    nc.scalar.activation(out=dst.bitcast(F32R), in_=zt,
                                         func=lrelu_func, alpha=NEG_SLOPE)
                else:
                    nc.vector.tensor_copy(out=dst.bitcast(F32R), in_=zt)
                    nc.vector.scalar_tensor_tensor(
                        out=dst.bitcast(F32R), in0=dst, scalar=NEG_SLOPE,
                        in1=dst, op0=AX.mult, op1=AX.max)
                if m % EB == EB - 1:
                    nc.scalar.activation(out=et.bitcast(F32R), in_=et,
                                         func=mybir.ActivationFunctionType.Exp)
                    for mu in range(m - EB + 1, m + 1):
                        b2 = (mu % EB) * 2048
                        u_mms(mu, et[:, b2:b2 + 2048],
                              mu == 0, mu == n_m - 1)
                    ets.pop(m // EB)

            emit_z(0)
            for m in range(n_m):
                if m + 1 < n_m:
                    emit_z(m + 1)
                process(m)
            finish()

        # ---------- layer0 attention -> x1T [64, h, n] (pre-gelu) ----------
        x1T = sb.tile([64, H0, N], F32)

        st1 = small.tile([64, 4, 6], F32, tag="st")
        mv1 = small.tile([64, H0, 2], F32, tag="mv1")

        for j in range(2):
            for half in range(2):
                # per-head U accumulators [65, 1024] (2 PSUM banks each)
                u_ps = [ps.tile([65, 1024], F32, tag="u", bufs=2,
                                name=f"u{j}{half}{s}") for s in range(2)]

                def z_mms(m, zt, j=j, half=half):
                    for s in range(2):
                        for q in range(2):
                            n_off = half * 1024 + q * 512
                            _mmr(nc, zt[:, s * 1024 + q * 512:
                                        s * 1024 + (q + 1) * 512],
                                 a0_sb[64 * s:64 * s + 64, j,
                                       m * 128:(m + 1) * 128],
                                 hpT[64 * s:64 * s + 64, j,
                                     n_off:n_off + 512])

                def u_mms(m, et_sl, start, stop, j=j, u_ps=u_ps):
                    for s in range(2):
                        for q in range(2):
                            _mmr(nc, u_ps[s][:, q * 512:(q + 1) * 512],
                                 hp1[:, 2 * j + s, m, :],
                                 et_sl[:, s * 1024 + q * 512:
                                       s * 1024 + (q + 1) * 512],
                                 start=start, stop=stop)

                def finish(j=j, half=half, u_ps=u_ps):
                    # denominator row goes PSUM -> DRAM -> 64-partition
                    # broadcast; reciprocal runs on the broadcast (the
                    # custom DVE recip only works partition-0-based on HW).
                    # U itself is copied out right away to free the PSUM
                    # accumulators for the next pass.
                    u_sb = ep.tile([65, 2, 1024], F32, tag="u_sb", bufs=1)
                    for s in range(2):
                        nc.vector.tensor_copy(out=u_sb[:, s, :], in_=u_ps[s])
                    r1_dram = dram.tile([1, 2, 1024], F32, tag="r1d")
                    nc.sync.dma_start(
                        out=r1_dram,
                        in_=u_sb[64:65, :, :])
                    rb_sb = ep.tile([64, 2, 1024], F32, tag="rb", bufs=1)
                    bc_ap = bass.AP(tensor=r1_dram.tensor,
                                    offset=r1_dram.offset,
                                    ap=[[0, 64], [1, 2048]])
                    nc.sync.dma_start(
                        out=rb_sb.rearrange("p a b -> p (a b)"), in_=bc_ap)
                    nc.vector.reciprocal_approx_fast(
                        out=rb_sb.rearrange("p a b -> p (a b)"),
                        in_=rb_sb.rearrange("p a b -> p (a b)"))
                    dst = x1T[:, 2 * j:2 * j + 2,
                              half * 1024:(half + 1) * 1024]
                    nc.vector.tensor_mul(out=dst.bitcast(F32R),
                                         in0=u_sb[0:64, :, :], in1=rb_sb)

                attention(z_mms, u_mms, NT, finish)

            # gelu(x + bias0) for this head pair; overlap stats with the
            # next pair's attention stream.
            nc.scalar.activation(out=x1T[:, 2 * j:2 * j + 2, :].bitcast(F32R),
                                 in_=x1T[:, 2 * j:2 * j + 2, :],
                                 func=gelu_func, bias=b0_sb)
            for h in (2 * j, 2 * j + 1):
                for c in range(4):
                    nc.vector.bn_stats(out=st1[:, c, :],
                                       in_=x1T[:, h, c * 512:(c + 1) * 512])
                nc.vector.bn_aggr(out=mv1[:, h, :], in_=st1)

        if taps:
            nc.sync.dma_start(out=taps["d_x1g"].ap(), in_=x1T)
        # ---------- instance norm 1 (per channel = (h, o)) ----------
        sd1 = small.tile([64, H0], F32, tag="sd1")
        rs1 = small.tile([64, H0], F32, tag="rs1")
        nc.scalar.activation(out=sd1, in_=mv1[:, :, 1],
                             func=mybir.ActivationFunctionType.Sqrt,
                             bias=eps_sb[0:64, :])
        nc.vector.reciprocal(out=rs1, in_=sd1)
        for h in range(H0):
            nc.vector.tensor_scalar(out=x1T[:, h, :].bitcast(F32R),
                                    in0=x1T[:, h, :],
                                    scalar1=mv1[:, h, 0:1],
                                    scalar2=rs1[:, h:h + 1],
                                    op0=AX.subtract, op1=AX.mult)

        # ---------- layer1 h': hpT1 duplicated on both partition halves ----
        hpT1 = sb.tile([128, N], F32)
        hp_ps1 = ps.tile([128, 2048], F32, tag="z", bufs=1)
        for kh in range(4):
            for c in range(4):
                _mmr(nc, hp_ps1[:, c * 512:(c + 1) * 512],
                     w1_sb[:, kh, :], x1T[:, kh, c * 512:(c + 1) * 512],
                     start=(kh == 0), stop=(kh == 3))
        nc.vector.tensor_copy(out=hpT1.bitcast(F32R), in_=hp_ps1)

        hp11 = sb.tile([128, NT, 65], F32)
        nc.vector.tensor_copy(out=hp11[:, :, 64].bitcast(F32R),
                              in_=ones_sb[:, 0:NT])
        for g in range(2):
            hpr_ps1 = ps.tile([128, 8, FO], F32, tag="z", bufs=1)
            for k in range(8):
                m = 8 * g + k
                nc.tensor.transpose(hpr_ps1[:, k, :],
                                    hpT1[0:64, m * 128:(m + 1) * 128],
                                    ident[0:64, 0:64])
            nc.vector.tensor_copy(
                out=hp11[:, 8 * g:8 * g + 8, 0:64].bitcast(F32R),
                in_=hpr_ps1)

        if taps:
            nc.sync.dma_start(out=taps["d_x1T"].ap(), in_=x1T)
            nc.sync.dma_start(out=taps["d_hpT1"].ap(), in_=hpT1)
            nc.sync.dma_start(out=taps["d_hp11"].ap(), in_=hp11)
        # ---------- layer1 attention (m-tile pairs) -> out ----------
        out_sb = sb.tile([128, NT, FO], F32)

        for half in range(2):
            u1_ps = ps.tile([65, 1024], F32, tag="u", bufs=2)

            def z_mms1(m2, zt, half=half):
                for s in range(2):
                    m = 2 * m2 + s
                    for q in range(2):
                        n_off = half * 1024 + q * 512
                        _mmr(nc, zt[:, s * 1024 + q * 512:
                                    s * 1024 + (q + 1) * 512],
                             a1_sb[64 * s:64 * s + 64, m * 128:(m + 1) * 128],
                             hpT1[64 * s:64 * s + 64, n_off:n_off + 512])

            def u_mms1(m2, et_sl, start, stop, u1_ps=u1_ps):
                for s in range(2):
                    for q in range(2):
                        _mmr(nc, u1_ps[:, q * 512:(q + 1) * 512],
                             hp11[:, 2 * m2 + s, :],
                             et_sl[:, s * 1024 + q * 512:
                                   s * 1024 + (q + 1) * 512],
                             start=(start and s == 0), stop=(stop and s == 1))

            def finish1(half=half, u1_ps=u1_ps):
                u1 = ep.tile([65, 1024], F32, tag="u_sb", bufs=1)
                nc.vector.tensor_copy(out=u1, in_=u1_ps)
                for g in range(2):
                    tr_ps = ps.tile([128, 4, 65], F32, tag="u", bufs=2)
                    for k in range(4):
                        t = 4 * g + k
                        nc.tensor.transpose(tr_ps[:, k, :],
                                            u1[:, t * 128:(t + 1) * 128],
                                            ident[0:65, 0:65])
                    rr = small.tile([128, 4], F32, tag="rr")
                    nc.vector.reciprocal(out=rr, in_=tr_ps[:, :, 64])
                    for k in range(4):
                        t = half * 8 + 4 * g + k
                        nc.vector.tensor_scalar(
                            out=out_sb[:, t, :], in0=tr_ps[:, k, 0:64],
                            scalar1=rr[:, k:k + 1], scalar2=None,
                            op0=AX.mult)

            attention(z_mms1, u_mms1, NT // 2, finish1)

        nc.sync.dma_start(out=out_d.ap().rearrange("(t p) f -> p t f", p=128),
                          in_=out_sb)


def _prep_host(inputs):
    """Host-side packing of weights into device layouts (replicated)."""
    f32 = np.float32
    asum0 = (np.asarray(inputs['a_src0'], f32)
             + np.asarray(inputs['a_dst0'], f32))        # [4, 64, n]
    a0 = np.empty((128, 2, N), f32)
    for h in range(H0):
        a0[64 * (h % 2):64 * (h % 2) + 64, h // 2, :] = asum0[h]
    w0r = np.asarray(inputs['w0'], f32)                  # [4, 64, 64]
    w0 = np.empty((64, 2, 128), f32)
    for j in range(2):
        w0[:, j, 0:64] = w0r[2 * j]
        w0[:, j, 64:128] = w0r[2 * j + 1]
    b0 = np.ascontiguousarray(np.asarray(inputs['bias0'], f32).reshape(64, 1))
    asum1 = (np.asarray(inputs['a_src1'], f32)
             + np.asarray(inputs['a_dst1'], f32))[0]     # [64, n]
    a1 = np.concatenate([asum1, asum1], axis=0)          # [128, n] dup
    w1r = np.asarray(inputs['w1'], f32)[0].reshape(4, 64, FO)
    w1 = np.empty((64, 4, 128), f32)
    for kh in range(4):
        w1[:, kh, 0:64] = w1r[kh]
        w1[:, kh, 64:128] = w1r[kh]
    return {'a0': np.ascontiguousarray(a0),
            'w0': np.ascontiguousarray(w0), 'b0': b0,
            'a1': np.ascontiguousarray(a1),
            'w1': np.ascontiguousarray(w1)}


_NC_CACHE = {}


def _get_nc(sim_safe=False):
    if sim_safe not in _NC_CACHE:
        _NC_CACHE[sim_safe] = build_bass(sim_safe=sim_safe)
    return _NC_CACHE[sim_safe]


LAST_RESULTS = None  # BassKernelResults of the last kernel() call


def kernel(**inputs):
    from concourse.bass_utils import run_bass_kernel_spmd
    global LAST_RESULTS

    nc = _get_nc(sim_safe=False)
    w = _prep_host(inputs)
    x = np.asarray(inputs['x'], dtype=np.float32)
    in_maps = [{'x': np.ascontiguousarray(x[i]), **w} for i in range(N_CORES)]

    res = run_bass_kernel_spmd(nc, in_maps, core_ids=list(range(N_CORES)))
    LAST_RESULTS = res
    out = np.stack([res.results[i]['out'] for i in range(N_CORES)])
    out = out + np.asarray(inputs['bias1'], dtype=np.float32)[None, None, :]
    return out.astype(np.float32)
